# revision 48
# baseline (speedup 1.0000x reference)
"""Trainium2 Bass kernel for nn_Cross_Domain_Class_Alignment.

Reference computation (per sample b):
    mask0[b] = argmin_k || feature_s2t[b,:,r,c] - centroid_target[k] ||^2
    mask1[b] = argmin_k || feature_target[b,:,r,c] - centroid_s2t[k] ||^2
    both nearest-upsampled from (65,129) to (512,1024), int32.

Sharding: data-parallel over batch B=8 across 8 NeuronCores (1 sample/core).
Centroids are replicated.

Per-core dataflow (per mask), software-pipelined per 2048-px quad:
  - features [256, 8385] streamed in 2048-px quads x 2 channel chunks
  - dist matmuls, centroid-stationary: psum quad [128, 512] holds four
    512-px banks at partition offsets {0,32,64,96} via tile_position
  - scalar ACT fuses m = 2*dots - csq (per-partition bias) PSUM->SBUF
  - PE transposes flip pixels onto partitions -> DVE argmin (reduce_max /
    is_ge / *(19-k) / reduce_max, first-index tie-break) per 16-block piece
  - per-quad [16,128] PE transpose + DRAM bounce reshapes flat pixel
    order into overlapping [17,129] row tiles (mrow_n)
  - row nearest-upsample: one-hot gather matmul g4^T @ mrow_n -> po
    [128,129] fp32 per 128-row output chunk (contract over 17 rows)
  - column nearest-upsample 129->1024 is folded into the int8 convert:
    3 DVE tensor_scalar ops with stride-0 read / stride-127 write APs
    (rep pattern: rep-7 at source cols 16,32,...,128; rep-8 elsewhere)
  - outputs written as int8 (classes 0..18), host casts to int32
All finish stages are emitted interleaved with the load stream in
readiness order so the post-stream tail is only the last piece chain.
"""

import numpy as np

B, C, h, w = 8, 256, 65, 129
K = 19
H, W = 512, 1024
HW = h * w              # 8385
QUAD_PX = 2048          # four 512-px banks per psum quad
NFULL = HW // QUAD_PX   # 4 full quads
REM = HW - NFULL * QUAD_PX   # 193 remainder pixels
NT = (HW + 127) // 128  # 66 pixel blocks of 128 (for the block matrix)


def _g4_onehot():
    """g4[s, 128n+i] = 1.0 iff floor((128n+i)*65/512) == 16n+s; [17,512] bf16.

    Output row chunk n (rows 128n..128n+127) sources rows 16n..16n+16 only,
    so each chunk's one-hot lives on partitions 0..16.
    """
    import ml_dtypes

    ri = (np.arange(H) * h) // H
    g4 = np.zeros((17, H), dtype=np.float32)
    for n in range(4):
        for i in range(128):
            s = ri[128 * n + i] - 16 * n
            assert 0 <= s < 17
            g4[s, 128 * n + i] = 1.0
    return g4.astype(ml_dtypes.bfloat16)


def build_module(num_devices=8):
    import concourse.bass as bass
    import concourse.tile as tile
    from concourse import bacc, mybir
    from concourse.ap import AP

    f32 = mybir.dt.float32
    bf16 = mybir.dt.bfloat16
    i8 = mybir.dt.int8

    nc = bacc.Bacc(
        "TRN2",
        target_bir_lowering=False,
        debug=False,
        enable_asserts=False,
        num_devices=num_devices,
    )

    f_s2t = nc.dram_tensor("feature_s2t", [C, HW], f32, kind="ExternalInput")
    f_tgt = nc.dram_tensor("feature_target", [C, HW], f32, kind="ExternalInput")
    c_s2t = nc.dram_tensor("centroid_s2t", [K, C], f32, kind="ExternalInput")
    c_tgt = nc.dram_tensor("centroid_target", [K, C], f32, kind="ExternalInput")
    out0 = nc.dram_tensor("out0", [H, W], i8, kind="ExternalOutput")
    out1 = nc.dram_tensor("out1", [H, W], i8, kind="ExternalOutput")

    import ml_dtypes

    ident_dram = nc.inline_tensor(np.eye(128, dtype=np.float32), name="ident_const")
    identb_dram = nc.inline_tensor(
        np.eye(128, dtype=ml_dtypes.bfloat16), name="identb_const"
    )
    g4_dram = nc.inline_tensor(_g4_onehot(), name="rowgather_const")
    wk_np = np.tile((K - np.arange(K)).astype(np.float32), (128, 1))
    wk_dram = nc.inline_tensor(wk_np, name="wk_const")
    # sel[k, 32j+k] = -1.0: replicates -csq over the four 32-partition groups
    sel_np = np.zeros((K, 128), dtype=np.float32)
    for j in range(4):
        sel_np[np.arange(K), 32 * j + np.arange(K)] = -1.0
    sel_dram = nc.inline_tensor(sel_np, name="sel_const")

    X = mybir.AxisListType.X
    ALU = mybir.AluOpType
    AF = mybir.ActivationFunctionType

    with tile.TileContext(nc) as tc:
        from contextlib import ExitStack

        with ExitStack() as ctx:
            const_p = ctx.enter_context(tc.tile_pool(name="const", bufs=1))
            feat_p = ctx.enter_context(tc.tile_pool(name="feat", bufs=6))
            q_p = ctx.enter_context(tc.tile_pool(name="q", bufs=3))
            s_p = ctx.enter_context(tc.tile_pool(name="s", bufs=2))
            pt_p = ctx.enter_context(tc.tile_pool(name="pt", bufs=2))
            m_p = ctx.enter_context(tc.tile_pool(name="m", bufs=2))
            oi_p = ctx.enter_context(tc.tile_pool(name="oi", bufs=3))
            ps_dist = ctx.enter_context(tc.tile_pool(name="psd", bufs=4, space="PSUM"))
            ps_tr = ctx.enter_context(tc.tile_pool(name="pst", bufs=2, space="PSUM"))
            ps_out = ctx.enter_context(tc.tile_pool(name="pso", bufs=2, space="PSUM"))
            dram_p = ctx.enter_context(tc.tile_pool(name="dram", bufs=2, space="DRAM"))

            # ---- constants on the FAST queues (sync/gpsimd), before the
            # feature ramp: the scalar queue delivers only a few GB/s and
            # would gate the prep chain by ~15us.
            cent_sbs = {}
            for pidx, cdram in ((0, c_tgt), (1, c_s2t)):
                cs = const_p.tile([K, C], f32, tag=f"cent{pidx}", name=f"cent_sb{pidx}")
                nc.sync.dma_start(out=cs[:], in_=cdram[:, :])
                cent_sbs[pidx] = cs
            ident = const_p.tile([128, 128], f32, tag="ident")
            nc.sync.dma_start(out=ident[:], in_=ident_dram[:, :])
            sel_sb = const_p.tile([K, 128], f32, tag="sel")
            nc.sync.dma_start(out=sel_sb[:], in_=sel_dram[:, :])
            wk_sb = const_p.tile([128, K], f32, tag="wk")
            nc.scalar.dma_start(out=wk_sb[:], in_=wk_dram[:, :])
            g4_sb = const_p.tile([17, H], bf16, tag="g4")
            nc.scalar.dma_start(out=g4_sb[:], in_=g4_dram[:, :])
            identb = const_p.tile([128, 128], bf16, tag="identb")
            nc.scalar.dma_start(out=identb[:], in_=identb_dram[:, :])

            # ---- feature loads: ONE DMA per quad, both channel chunks into
            # a single [128, 2*QUAD_PX] tile (src AP [cc, part, px]); all on
            # sync's queue (the only one that sustains full HBM bandwidth).
            # Fewer DMAs also keeps the DMA-sem ring short so tail DMAs
            # don't chain behind late feature loads.
            def load_quad(feat, q):
                px0 = q * QUAD_PX
                pxw = QUAD_PX if q < NFULL else HW - px0
                ft = feat_p.tile([128, 2 * QUAD_PX], f32, tag="feat")
                nc.sync.dma_start(
                    out=ft[:, 0 : 2 * pxw].rearrange("p (c x) -> p c x", c=2),
                    in_=feat[:, px0 : px0 + pxw]
                    .rearrange("(c p) x -> p c x", c=2),
                )
                return ft

            fg_ramp = {0: load_quad(f_s2t, 0), 1: load_quad(f_tgt, 0)}

            # ---- per-pair centroid prep ----
            def prep_pair(pidx):
                cent_sb = cent_sbs[pidx]
                # centT first: it gates the dist matmuls (csqn4 only gates
                # the later ACT).  centT chunks [128, 32]: cols 0:19 =
                # cent^T, cols 19:32 = 0
                centT = []
                for cc in range(2):
                    ct = const_p.tile([128, 32], f32, tag=f"centT{pidx}_{cc}")
                    nc.vector.memset(ct[:], 0.0)
                    pt = ps_tr.tile([128, K], f32, tag="tr")
                    nc.tensor.transpose(
                        pt[:], cent_sb[:, cc * 128 : (cc + 1) * 128], ident[:K, :K]
                    )
                    nc.vector.tensor_copy(out=ct[:, 0:K], in_=pt[:])
                    centT.append(ct)
                sq = const_p.tile([K, C], f32, tag=f"centsq{pidx}")
                nc.vector.tensor_mul(sq[:], cent_sb[:], cent_sb[:])
                csq = const_p.tile([K, 1], f32, tag=f"csq{pidx}")
                nc.vector.reduce_sum(csq[:], sq[:], axis=X)
                # -csq replicated at partition offsets {0,32,64,96}
                pb = ps_tr.tile([128, 1], f32, tag="tr")
                nc.tensor.matmul(pb[:], sel_sb[:], csq[:], start=True, stop=True)
                csqn4 = const_p.tile([128, 1], f32, tag=f"csqn4_{pidx}")
                nc.vector.tensor_copy(out=csqn4[:], in_=pb[:])
                return centT, csqn4

            class MaskCtx:
                pass

            def make_mc(feat, out_dram):
                mc = MaskCtx()
                mc.feat, mc.out_dram = feat, out_dram
                # sg layout: value for pixel block b (= p//128), class k at
                # column 19*b + k  (66 blocks x 19 = 1254, padded)
                mc.sg = s_p.tile([128, NT * K + 40], f32, tag="s")
                mc.ptf = pt_p.tile([128, NT], bf16, tag="ptf")
                mc.mx = pt_p.tile([128, NT], f32, tag="mx")
                mc.scratch = dram_p.tile([NT, 128], bf16, tag="scratch")
                mc.mrow = {}
                mc.quad = {}
                return mc

            def mm_q(mc, q, ft):
                """dist matmuls + ACT for full quad q."""
                psq = ps_dist.tile([128, 512], f32, tag="dist")
                for j in range(4):
                    for cc in range(2):
                        nc.tensor.matmul(
                            psq[32 * j : 32 * j + 32, :],
                            mc.centT[cc][:],
                            ft[:, QUAD_PX * cc + 512 * j : QUAD_PX * cc + 512 * j + 512],
                            start=(cc == 0),
                            stop=(cc == 1),
                            tile_position=(0, 32 * j),
                        )
                quad = q_p.tile([128, 512], f32, tag="quad")
                nc.scalar.activation(
                    out=quad[:],
                    in_=psq[:],
                    func=AF.Identity,
                    bias=mc.csqn4[:],
                    scale=2.0,
                )
                mc.quad[q] = quad

            def tr_q(mc, q):
                """transposes + sg copy for full quad q (1 slot after mm_q)."""
                quad = mc.quad.pop(q)
                ptr4 = ps_tr.tile([128, 512], f32, tag="tr")
                for tq in range(4):
                    nc.tensor.transpose(
                        ptr4[:, 128 * tq : 128 * tq + 128],
                        quad[:, 128 * tq : 128 * tq + 128],
                        ident[:],
                    )
                # ptr4 col = 128*tq + 32*j + k'; block b = 16*q + 4*j + tq
                base = K * 16 * q
                nc.vector.tensor_copy(
                    out=mc.sg[:, base : base + 16 * K].rearrange(
                        "p (j tq k) -> p j tq k", tq=4, k=K
                    ),
                    in_=ptr4[:]
                    .rearrange("p (tq j e) -> p tq j e", j=4, e=32)[:, :, :, 0:K]
                    .transpose([0, 2, 1, 3]),
                )

            def mm_rem(mc, ft):
                """remainder: 193 px, single 32-partition group, blocks 64-65."""
                px0 = NFULL * QUAD_PX
                pxw = HW - px0
                psr = ps_dist.tile([32, 256], f32, tag="dist")
                nc.vector.memset(psr[:, pxw:256], 0.0)
                for cc in range(2):
                    nc.tensor.matmul(
                        psr[0:32, 0:pxw],
                        mc.centT[cc][:],
                        ft[:, pxw * cc : pxw * cc + pxw],
                        start=(cc == 0),
                        stop=(cc == 1),
                    )
                st2 = q_p.tile([32, 256], f32, tag="st2")
                nc.scalar.activation(
                    out=st2[:],
                    in_=psr[:],
                    func=AF.Identity,
                    bias=mc.csqn4[0:32, :],
                    scale=2.0,
                )
                mc.strem = st2

            def tr_rem(mc):
                st2 = mc.strem
                for tq in range(2):
                    b = 64 + tq
                    ptr = ps_tr.tile([128, 32], f32, tag="tr")
                    nc.tensor.transpose(
                        ptr[:], st2[:, 128 * tq : 128 * tq + 128], ident[:32, :32]
                    )
                    nc.vector.tensor_copy(
                        out=mc.sg[:, K * b : K * b + K],
                        in_=ptr[:, 0:K],
                    )

            def am(mc, b0, b1):
                """argmin blocks [b0,b1): y = 19 - argmin over k, first-index."""
                nb = b1 - b0
                sl = mc.sg[:, K * b0 : K * b1].rearrange("p (b k) -> p b k", k=K)
                mxs = mc.mx[:, b0:b1]
                nc.vector.tensor_reduce(mxs, sl, axis=X, op=ALU.max)
                eq = s_p.tile([128, 16 * K], f32, tag="eq")
                eqs = eq[:, 0 : nb * K].rearrange("p (b k) -> p b k", k=K)
                nc.vector.tensor_tensor(
                    out=eqs,
                    in0=sl,
                    in1=mxs.unsqueeze(2).broadcast_to([128, nb, K]),
                    op=ALU.is_ge,
                )
                nc.vector.tensor_tensor(
                    out=eqs,
                    in0=eqs,
                    in1=wk_sb[:].unsqueeze(1).broadcast_to([128, nb, K]),
                    op=ALU.mult,
                )
                nc.vector.tensor_reduce(mc.ptf[:, b0:b1], eqs, axis=X, op=ALU.max)

            def ptsc(mc, b0, b1):
                """block matrix -> flat pixel order, rows [b0,b1) of scratch."""
                nb = b1 - b0
                ptt = ps_out.tile([nb, 128], bf16, tag="po")
                nc.tensor.transpose(ptt[:], mc.ptf[:, b0:b1], identb[:])
                pttsb = pt_p.tile([nb, 128], bf16, tag="pttsb")
                nc.scalar.activation(out=pttsb[:], in_=ptt[:], func=AF.Copy)
                nc.gpsimd.dma_start(out=mc.scratch[b0:b1, :], in_=pttsb[:])

            def md(mc, n):
                """DRAM bounce read: mrow_n[s,c] = y at pixel 129*(16n+s)+c."""
                mr = m_p.tile([17, w], bf16, tag=f"mrow{n}")
                mc.mrow[n] = mr
                nc.gpsimd.dma_start(
                    out=mr[:],
                    in_=mc.scratch[:]
                    .rearrange("a b -> (a b)")[w * 16 * n : w * (16 * n + 17)]
                    .rearrange("(r c) -> r c", c=w),
                )

            def gacvod(mc, n):
                """row-gather matmul + fused column-expand int8 convert + store."""
                po = ps_out.tile([128, w], f32, tag="po")
                nc.tensor.matmul(
                    po[:],
                    g4_sb[:, n * 128 : (n + 1) * 128],
                    mc.mrow[n][:],
                    start=True,
                    stop=True,
                )
                oint = oi_p.tile([128, W], i8, tag="oint")
                # opA: src cols 0..15, rep 8 -> out cols 0..127
                nc.vector.tensor_scalar(
                    out=oint[:, 0:128].rearrange("p (s r) -> p s r", r=8),
                    in0=po[:, 0:16].unsqueeze(2).broadcast_to([128, 16, 8]),
                    scalar1=-1.0,
                    scalar2=float(K),
                    op0=ALU.mult,
                    op1=ALU.add,
                )
                # opB: src cols 16,32,...,128 (8 cols), rep 7 ->
                #      out cols 128+127j .. 128+127j+6
                outB = oint[:, 128:]
                ppair = [int(outB.ap[0][0]), int(outB.ap[0][1])]
                nc.vector.tensor_scalar(
                    out=AP(
                        tensor=outB.tensor,
                        offset=outB.offset,
                        ap=[ppair, [127, 8], [1, 7]],
                    ),
                    in0=po[:, 16:129:16].unsqueeze(2).broadcast_to([128, 8, 7]),
                    scalar1=-1.0,
                    scalar2=float(K),
                    op0=ALU.mult,
                    op1=ALU.add,
                )
                # opC: src cols 17+16j+s (j<7, s<15), rep 8 ->
                #      out cols 135+127j+8s .. +7; j split vector/scalar
                outC = oint[:, 135:]
                inC = (
                    po[:, 17:129]
                    .rearrange("p (j x) -> p j x", x=16)[:, :, 0:15]
                    .unsqueeze(3)
                    .broadcast_to([128, 7, 15, 8])
                )
                nc.vector.tensor_scalar(
                    out=AP(
                        tensor=outC.tensor,
                        offset=outC.offset,
                        ap=[ppair, [127, 4], [8, 15], [1, 8]],
                    ),
                    in0=inC[:, 0:4],
                    scalar1=-1.0,
                    scalar2=float(K),
                    op0=ALU.mult,
                    op1=ALU.add,
                )
                outC2 = oint[:, 135 + 4 * 127 :]
                nc.scalar.activation(
                    out=AP(
                        tensor=outC2.tensor,
                        offset=outC2.offset,
                        ap=[ppair, [127, 3], [8, 15], [1, 8]],
                    ),
                    in_=inC[:, 4:7],
                    func=AF.Copy,
                    bias=float(K),
                    scale=-1.0,
                )
                nc.sync.dma_start(
                    out=mc.out_dram[n * 128 : (n + 1) * 128, :],
                    in_=oint[:],
                )

            mcA = make_mc(f_s2t, out0)
            mcB = make_mc(f_tgt, out1)
            A, Bm = mcA, mcB

            # ---- software-pipelined emission, readiness order ----
            fgA0, fgB0 = fg_ramp[0], fg_ramp[1]

            A.centT, A.csqn4 = (ct := prep_pair(0))[0], ct[1]   # mask0: tgt
            Bm.centT, Bm.csqn4 = (ct := prep_pair(1))[0], ct[1]  # mask1: s2t

            def quad_slot(mc, q, pieces, tr=None, fg=None):
                """One load-slot: loads, ripe tail pieces, deferred
                transposes from the previous slot, this slot's matmuls."""
                if fg is None:
                    fg = load_quad(mc.feat, q)
                for p in pieces:
                    p()
                if tr:
                    tr()
                if q < NFULL:
                    mm_q(mc, q, fg)
                else:
                    mm_rem(mc, fg)

            # Interleaved A/B quads; rem quads early so they leave the tail.
            # Transposes run one slot after their matmuls (hides the scalar
            # ACT roundtrip off the tensor FIFO); other tail pieces are
            # emitted >=1-2 load-slots after their producers, so per-engine
            # FIFO order matches data-readiness order.
            quad_slot(A, 0, [], fg=fgA0)                                 # L0
            quad_slot(Bm, 0, [], tr=lambda: tr_q(A, 0), fg=fgB0)         # L1
            quad_slot(A, 4, [lambda: am(A, 0, 16)],
                      tr=lambda: tr_q(Bm, 0))                            # L2
            quad_slot(Bm, 4, [lambda: am(Bm, 0, 16),
                              lambda: ptsc(A, 0, 16)],
                      tr=lambda: tr_rem(A))                              # L3
            quad_slot(A, 1, [lambda: am(A, 64, 66),
                             lambda: ptsc(Bm, 0, 16)],
                      tr=lambda: tr_rem(Bm))                             # L4
            quad_slot(Bm, 1, [lambda: am(Bm, 64, 66),
                              lambda: ptsc(A, 64, 66)],
                      tr=lambda: tr_q(A, 1))                             # L5
            quad_slot(A, 2, [lambda: am(A, 16, 32),
                             lambda: ptsc(A, 16, 32),
                             lambda: ptsc(Bm, 64, 66)],
                      tr=lambda: tr_q(Bm, 1))                            # L6
            quad_slot(Bm, 2, [lambda: am(Bm, 16, 32),
                              lambda: ptsc(Bm, 16, 32),
                              lambda: md(A, 0)],
                      tr=lambda: tr_q(A, 2))                             # L7
            quad_slot(A, 3, [lambda: am(A, 32, 48),
                             lambda: ptsc(A, 32, 48),
                             lambda: md(Bm, 0),
                             lambda: gacvod(A, 0)],
                      tr=lambda: tr_q(Bm, 2))                            # L8
            quad_slot(Bm, 3, [lambda: am(Bm, 32, 48),
                              lambda: ptsc(Bm, 32, 48),
                              lambda: md(A, 1),
                              lambda: gacvod(Bm, 0)],
                      tr=lambda: tr_q(A, 3))                             # L9
            # ---- post-stream drain (deps land in this order) ----
            tr_q(Bm, 3)
            am(A, 48, 64)
            ptsc(A, 48, 64)
            md(Bm, 1)
            gacvod(A, 1)
            am(Bm, 48, 64)
            ptsc(Bm, 48, 64)
            md(A, 2)
            md(A, 3)
            gacvod(Bm, 1)
            md(Bm, 2)
            md(Bm, 3)
            gacvod(A, 2)
            gacvod(A, 3)
            gacvod(Bm, 2)
            gacvod(Bm, 3)

    nc.compile()
    return nc


_cached_nc = None


def _get_nc():
    global _cached_nc
    if _cached_nc is None:
        _cached_nc = build_module()
    return _cached_nc


def make_in_maps(feature_s2t, feature_target, centroid_s2t, centroid_target):
    in_maps = []
    for b in range(B):
        in_maps.append(
            {
                "feature_s2t": np.ascontiguousarray(
                    feature_s2t[b], dtype=np.float32
                ).reshape(C, HW),
                "feature_target": np.ascontiguousarray(
                    feature_target[b], dtype=np.float32
                ).reshape(C, HW),
                "centroid_s2t": np.ascontiguousarray(centroid_s2t, dtype=np.float32),
                "centroid_target": np.ascontiguousarray(
                    centroid_target, dtype=np.float32
                ),
            }
        )
    return in_maps


def kernel(
    feature_s2t,
    feature_target,
    centroid_s2t,
    centroid_target,
    seg_s2t=None,
    seg_target=None,
    **_unused,
):
    from concourse.bass_utils import run_bass_kernel_spmd

    nc = _get_nc()
    in_maps = make_in_maps(
        np.asarray(feature_s2t),
        np.asarray(feature_target),
        np.asarray(centroid_s2t),
        np.asarray(centroid_target),
    )
    res = run_bass_kernel_spmd(nc, in_maps, core_ids=list(range(B)))
    results = res.results
    m0 = np.stack([results[b]["out0"] for b in range(B)]).astype(np.int32)
    m1 = np.stack([results[b]["out1"] for b in range(B)]).astype(np.int32)
    return (m0, m1)
